# revision 29
# baseline (speedup 1.0000x reference)
"""Causal self-attention (B=4, T=2048, C=1024, H=16) on 8 NeuronCores.

Sharding: core c handles batch b = c//2 and head-half half = c%2 (8 heads,
512 channels). QKV projections are column-parallel, output projection is
row-parallel (Megatron); the two per-batch bf16 output partials are summed
on host in fp32.

All matmul operands are bf16 (fp32 PSUM accumulate): same 1 cycle/row PE
stream rate as fp32r but with fast-weight-load, 2-4x DVE element ops, and
half the DMA/SBUF footprint. Tolerance is 2e-2; bf16 end-to-end measures
~1e-3.

Per-core pipeline:
  phase 1: qT/kT = W @ xT (weight-stationary, [m, t] layout; bias applied
           free on the ACT psum-evacuation via a per-partition bias),
           v = xT.T @ WvT ([t, m] layout; bias folded into the DVE
           evacuation) with a constant ones-column per head (vAug) so the
           attention matmul also produces the softmax denominator.
  phase 2: processes HEAD PAIRS (2p, 2p+1) = partitions 0-63/64-127 of the
           kT/qT/yT tile p, in 512-query chunks. The hd=64 score matmuls
           run as two concurrent 64x128 PE row tiles (T0/T8), halving score
           time; the AV matmul splits its 128-key contraction into T0
           (keys 0-63 -> psyA) and T8 (keys 64-127 -> psyB) so the whole
           phase stays in 64-row tiling mode (no PE drains). P = exp(S^T)
           via one strided ACT instruction per key block covering both
           heads' column spans; 0/1 mask multiply on diagonal blocks.
           Softmax uses a fixed max of 0 (scores are ~N(0,1)); the
           denominator is row 64 of psy. psyA+psyB merge on the DVE
           evacuation; normalize via reciprocal + ones-broadcast +
           multiply, writing yT straight into SBUF.
  phase 3: out_partial = yT.T @ WpT (bias folded into the DVE evacuation;
           bp on half==0 cores only), bf16 output.

Benching: with bench_loops and all three phases selected, ONE hardware
For_i loop wraps the whole pipeline so consecutive iterations overlap
(phase 2's scalar-engine exp burst hides under phase 3 / next phase 1 PE
work). PSUM budget (8 banks): ps_s 2x[128,1024]=4 shared by all phases'
matmul groups + psyA/psyB [65,1024] = 2+2. The PE streams ~1 col/cycle at
an effective ~1.2 GHz in this environment (measured; the warm 2.4 GHz
state never engages), so total PE columns (~471k/core/iter) set the
floor.
"""

import sys
import types

import numpy as np
from contextlib import ExitStack

import ml_dtypes

import concourse.bass as bass
import concourse.mybir as mybir
import concourse.tile as tile
from concourse import bacc
from concourse.bass_utils import run_bass_kernel_spmd

# If the environment sets BASS_TRACE but ships only the antenv stub (no
# axon_hooks), run_bass_kernel_spmd would crash on import. Provide the
# graceful "no hook registered" fallback only when the real module is absent.
try:  # pragma: no cover
    import antenv.axon_hooks  # noqa: F401
except ImportError:  # pragma: no cover
    import antenv

    _stub = types.ModuleType("antenv.axon_hooks")
    _stub.get_axon_ntff_profile_hook = lambda: None
    sys.modules["antenv.axon_hooks"] = _stub
    antenv.axon_hooks = _stub

F32 = mybir.dt.float32
BF16 = mybir.dt.bfloat16
EXP = mybir.ActivationFunctionType.Exp
IDENT = mybir.ActivationFunctionType.Identity
NP_BF16 = ml_dtypes.bfloat16

B, T, C, H = 4, 2048, 1024, 16
HD = C // H              # 64 head dim
N_CORES = 8
HPC = H // 2             # 8 heads per core
MPC = C // 2             # 512 channels per core
MT = MPC // 128          # 4 m-tiles per core
CT = C // 128            # 8 contraction tiles
TC = T // 512            # 4 t-chunks
TT = T // 128            # 16 t-tiles
SCALE = float(1.0 / np.sqrt(HD))

_CACHE = {}


def _build(bench_loops=None, phases=(1, 2, 3)):
    import contextlib

    def _rep(cond=True):
        return tc.For_i(0, bench_loops, 1) if (bench_loops and cond) else contextlib.nullcontext()

    fused = bool(bench_loops) and set(phases) == {1, 2, 3}

    nc = bacc.Bacc()
    xT = nc.declare_dram_parameter("xT", [C, T], BF16, isOutput=False)
    wqT = nc.declare_dram_parameter("wqT", [C, MPC], BF16, isOutput=False)
    wkT = nc.declare_dram_parameter("wkT", [C, MPC], BF16, isOutput=False)
    wvT = nc.declare_dram_parameter("wvT", [C, MPC], BF16, isOutput=False)
    wpT = nc.declare_dram_parameter("wpT", [MPC, C], BF16, isOutput=False)
    bqc = nc.declare_dram_parameter("bqc", [128, MT], F32, isOutput=False)
    bkc = nc.declare_dram_parameter("bkc", [128, MT], F32, isOutput=False)
    bv = nc.declare_dram_parameter("bv", [1, MPC], F32, isOutput=False)
    bp = nc.declare_dram_parameter("bp", [1, C], F32, isOutput=False)
    mask01d = nc.declare_dram_parameter("mask01", [128, 128], BF16, isOutput=False)
    outp = nc.declare_dram_parameter("out", [T, C], BF16, isOutput=True)

    with tile.TileContext(nc) as tc:
        with ExitStack() as ctx:
            persist = ctx.enter_context(tc.tile_pool(name="persist", bufs=1))
            pool_p = ctx.enter_context(tc.tile_pool(name="pool_p", bufs=2))
            # PSUM budget (8 banks): ps_s 2x[128,1024]=4 shared by all phases'
            # matmul groups; psyA+psyB [65,1024] = 2+2.
            ps_s = ctx.enter_context(tc.tile_pool(name="ps_s", bufs=2, space="PSUM"))
            ps_y = ctx.enter_context(tc.tile_pool(name="ps_y", bufs=1, space="PSUM"))

            # ---- constants / small tensors ----
            mask01_sb = persist.tile([128, 128], BF16, name="mask01_sb")
            nc.sync.dma_start(out=mask01_sb, in_=mask01d[:, :])
            bqc_sb = persist.tile([128, MT], F32, name="bqc_sb")
            nc.sync.dma_start(out=bqc_sb, in_=bqc[:, :])
            bkc_sb = persist.tile([128, MT], F32, name="bkc_sb")
            nc.sync.dma_start(out=bkc_sb, in_=bkc[:, :])
            bv_sb = persist.tile([1, MPC], F32, name="bv_sb")
            nc.sync.dma_start(out=bv_sb, in_=bv[:, :])
            bp_sb = persist.tile([1, C], F32, name="bp_sb")
            nc.sync.dma_start(out=bp_sb, in_=bp[:, :])
            ones8 = persist.tile([128, 8], BF16, name="ones8")
            nc.vector.memset(ones8, 1.0)
            # biases broadcast across partitions once (f32, matching the psum
            # operand dtype); folded into the DVE evacuations instead of
            # burning PE columns on rank-1 matmuls
            bv_bc = persist.tile([128, MPC], F32, name="bv_bc")
            nc.gpsimd.partition_broadcast(bv_bc, bv_sb)
            bp_bc = persist.tile([128, C], F32, name="bp_bc")
            nc.gpsimd.partition_broadcast(bp_bc, bp_sb)

            # ---- persistent activations ----
            qT_sb = [persist.tile([128, T], BF16, name=f"qT{m}") for m in range(MT)]
            kT_sb = [persist.tile([128, T], BF16, name=f"kT{m}") for m in range(MT)]
            vAug = [persist.tile([128, HPC * (HD + 1)], BF16, name=f"vAug{t}") for t in range(TT)]
            yT_sb = [persist.tile([128, T], BF16, name=f"yT{m}") for m in range(MT)]

            # constant ones-column of vAug, written once
            for t_ in range(TT):
                va = vAug[t_].rearrange("p (h w) -> p h w", w=HD + 1)
                nc.vector.tensor_copy(va[:, :, HD], ones8)

            if 1 not in phases:
                # bench-only: initialize phase-1 products with arbitrary finite data
                for m in range(MT):
                    nc.sync.dma_start(out=qT_sb[m], in_=xT[0:128, :])
                    nc.sync.dma_start(out=kT_sb[m], in_=xT[128:256, :])
                for t_ in range(TT):
                    va = vAug[t_].rearrange("p (h w) -> p h w", w=HD + 1)
                    nc.sync.dma_start(out=va[:, :, 0:HD],
                                      in_=xT[0:128, 0:HPC * HD].rearrange("p (h w) -> p h w", w=HD))
            if 3 in phases and 2 not in phases:
                for m in range(MT):
                    nc.sync.dma_start(out=yT_sb[m], in_=xT[0:128, :])

            # ---- weights in SBUF (loaded once, outside any bench loop) ----
            pool_w = ctx.enter_context(tc.tile_pool(name="pool_w", bufs=1))
            if 1 in phases:
                wq_t = [pool_w.tile([128, MPC], BF16, name=f"wq{c}") for c in range(CT)]
                wk_t = [pool_w.tile([128, MPC], BF16, name=f"wk{c}") for c in range(CT)]
                wv_t = [pool_w.tile([128, MPC], BF16, name=f"wv{c}") for c in range(CT)]
                for c in range(CT):
                    nc.sync.dma_start(out=wq_t[c], in_=wqT[c * 128:(c + 1) * 128, :])
                    nc.sync.dma_start(out=wk_t[c], in_=wkT[c * 128:(c + 1) * 128, :])
                    nc.sync.dma_start(out=wv_t[c], in_=wvT[c * 128:(c + 1) * 128, :])
            if 3 in phases:
                wp_t = [pool_w.tile([128, C], BF16, name=f"wp{m}") for m in range(MT)]
                for m in range(MT):
                    nc.sync.dma_start(out=wp_t[m], in_=wpT[m * 128:(m + 1) * 128, :])

            pool_xs = ctx.enter_context(tc.tile_pool(name="pool_xs", bufs=1))
            pool_p2 = ctx.enter_context(tc.tile_pool(name="pool_p2", bufs=3))
            pool_o = ctx.enter_context(tc.tile_pool(name="pool_o", bufs=1))

            # ================= phase bodies =================
            def body1():
                for tch in range(TC):
                    t0 = tch * 512
                    xs = []
                    for c in range(CT):
                        x_ = pool_xs.tile([128, 512], BF16, name=f"xs_{tch}_{c}", tag="xs", bufs=12)
                        nc.sync.dma_start(out=x_, in_=xT[c * 128:(c + 1) * 128, t0:t0 + 512])
                        xs.append(x_)
                    # qT / kT (weight-stationary): psum[m 128, t 512]; bias is
                    # per-partition so the DVE evacuation applies it for free.
                    for wt, bias_col, dst in ((wq_t, bqc_sb, qT_sb), (wk_t, bkc_sb, kT_sb)):
                        for m in range(MT):
                            ps = ps_s.tile([128, 512], F32, name=f"ps_{tch}_{id(wt)}_{m}",
                                           tag="pss", padded_shape=[128, 1024])
                            for c in range(CT):
                                nc.tensor.matmul(ps, wt[c][:, m * 128:(m + 1) * 128], xs[c],
                                                 start=(c == 0), stop=(c == CT - 1))
                            nc.scalar.activation(out=dst[m][:, t0:t0 + 512], in_=ps,
                                                 func=IDENT, bias=bias_col[:, m:m + 1])
                    # v (x-stationary): psum[t 128, m 512] -> vAug; bias folded
                    # into the DVE evacuation via the broadcast bias tile.
                    for tt in range(4):
                        tg = tch * 4 + tt
                        ps = ps_s.tile([128, MPC], F32, name=f"psv_{tg}",
                                       tag="pss", padded_shape=[128, 1024])
                        for c in range(CT):
                            nc.tensor.matmul(ps, xs[c][:, tt * 128:(tt + 1) * 128], wv_t[c],
                                             start=(c == 0), stop=(c == CT - 1))
                        va = vAug[tg].rearrange("p (h w) -> p h w", w=HD + 1)
                        nc.vector.tensor_add(va[:, :, 0:HD],
                                             ps.rearrange("p (h w) -> p h w", w=HD),
                                             bv_bc.rearrange("p (h w) -> p h w", w=HD))

            def body2():
                # Head PAIRS (2p, 2p+1) = partitions 0-63 / 64-127 of kT/qT tile
                # p. Scores run as two concurrent 64x128 row tiles (T0/T8); AV
                # splits the 128-key contraction into T0 (keys 0-63 -> psyA) and
                # T8 (keys 64-127 -> psyB), also concurrent. All phase-2 matmuls
                # stay in 64-row tiling mode (no PE drains inside the loop).
                # Query chunks of 512: psum layout per chunk is
                #   ps_pair[128, 1024]: cols 0:512 even head (bank 0),
                #                       cols 512:1024 odd head (bank 1)
                #   psyA/psyB[65, 1024]: cols 0:512 even (bank), 512:1024 odd
                #   (bank) — one start/stop per bank per accumulation round.
                W = 512
                for p in range(MT):
                    hE, hO = 2 * p, 2 * p + 1
                    kT_h = kT_sb[p]
                    qT_h = qT_sb[p]
                    for qc in range(T // W):
                        q0 = qc * W
                        jmax = 4 * qc + 3
                        psyA = ps_y.tile([HD + 1, 1024], F32, name=f"psyA_{p}_{qc}", tag="psyA")
                        psyB = ps_y.tile([HD + 1, 1024], F32, name=f"psyB_{p}_{qc}", tag="psyB")
                        pend = None

                        def _emit_y(j, P, lo):
                            # row groups alternate so each LDWEIGHTS hides under
                            # the other tile's in-flight matmul
                            for hh, co in ((hE, 0), (hO, W)):
                                for half, psy in ((0, psyA), (64, psyB)):
                                    nc.tensor.matmul(
                                        psy[:, co + lo:co + W],
                                        vAug[j][half:half + 64, hh * (HD + 1):(hh + 1) * (HD + 1)],
                                        P[half:half + 64, co + lo:co + W],
                                        start=(j == 0), stop=(j == jmax),
                                        tile_position=(half, 0))

                        for j in range(jmax + 1):
                            lo = max(0, 128 * j - q0)
                            pss = ps_s.tile([128, 1024], F32, name=f"pss_{p}_{qc}_{j}", tag="pss")
                            for half, co in ((0, 0), (64, W)):
                                nc.tensor.matmul(
                                    pss[:, co + lo:co + W],
                                    kT_h[half:half + HD, j * 128:(j + 1) * 128],
                                    qT_h[half:half + HD, q0 + lo:q0 + W],
                                    start=True, stop=True, tile_position=(half, 0))
                            P = pool_p.tile([128, 1024], BF16, name=f"P_{p}_{qc}_{j}", tag="P", bufs=4)
                            # one strided activation covers both heads' spans
                            pv = pss.rearrange("p (g w) -> p g w", w=W)
                            Pv = P.rearrange("p (g w) -> p g w", w=W)
                            nc.scalar.activation(out=Pv[:, :, lo:W], in_=pv[:, :, lo:W],
                                                 func=EXP, scale=SCALE)
                            if 128 * j >= q0:  # diagonal block: in-tile causal mask
                                for co in (0, W):
                                    nc.vector.tensor_mul(P[:, co + lo:co + lo + 128],
                                                         P[:, co + lo:co + lo + 128], mask01_sb)
                            if pend is not None:
                                _emit_y(*pend)
                            pend = (j, P, lo)
                        _emit_y(*pend)
                        # merge the two key-half accumulators (both PSUM, so two
                        # separate evacuations — DVE + ACT — then a cheap bf16
                        # add), normalize, and write yT straight into SBUF.
                        yuA = pool_p2.tile([HD + 1, 1024], BF16, name=f"yuA_{p}_{qc}", tag="yuA")
                        nc.vector.tensor_copy(yuA, psyA)
                        yuB = pool_p2.tile([HD + 1, 1024], BF16, name=f"yuB_{p}_{qc}", tag="yuB")
                        nc.vector.tensor_copy(yuB, psyB)
                        yu = pool_p2.tile([HD + 1, 1024], BF16, name=f"yu_{p}_{qc}", tag="yu")
                        nc.vector.tensor_add(yu, yuA, yuB)
                        r16 = pool_p2.tile([1, 1024], BF16, name=f"r16_{p}_{qc}", tag="r16")
                        with nc.allow_low_precision(reason="denominator recip; 2e-2 tol"):
                            nc.vector.reciprocal(r16, yu[HD:HD + 1, :])
                        rb = pool_p2.tile([HD, 1024], BF16, name=f"rb_{p}_{qc}", tag="rb")
                        nc.gpsimd.partition_broadcast(rb, r16)
                        for so, co in ((0, 0), (64, W)):
                            nc.vector.tensor_mul(yT_sb[p][so:so + HD, q0:q0 + W],
                                                 yu[0:HD, co:co + W], rb[:, co:co + W])

            def body3():
                for tt in range(TT):
                    for nch in range(2):
                        n0 = nch * 512
                        ps = ps_s.tile([128, 512], F32, name=f"pso_{tt}_{nch}",
                                       tag="pss", padded_shape=[128, 1024])
                        for m in range(MT):
                            nc.tensor.matmul(ps, yT_sb[m][:, tt * 128:(tt + 1) * 128],
                                             wp_t[m][:, n0:n0 + 512],
                                             start=(m == 0), stop=(m == MT - 1))
                        o_sb = pool_o.tile([128, 512], BF16, name=f"o_{tt}_{nch}", tag="o", bufs=3)
                        nc.vector.tensor_add(o_sb, ps, bp_bc[:, n0:n0 + 512])
                        nc.sync.dma_start(out=outp[tt * 128:(tt + 1) * 128, n0:n0 + 512], in_=o_sb)

            # ================= emission =================
            if fused:
                with _rep():
                    body1()
                    body2()
                    body3()
            else:
                if 1 in phases:
                    with _rep():
                        body1()
                if 2 in phases:
                    with _rep():
                        body2()
                if 3 in phases:
                    with _rep():
                        body3()
    nc.finalize()
    return nc


def _get_nc(bench_loops=None, phases=(1, 2, 3)):
    key = ("nc", bench_loops, tuple(phases))
    if key not in _CACHE:
        _CACHE[key] = _build(bench_loops, phases)
    return _CACHE[key]


def make_in_maps(x, Wk, bk, Wq, bq, Wv, bv, Wp, bp):
    x = np.asarray(x, dtype=np.float32)
    Wk, Wq, Wv, Wp = (np.asarray(a, dtype=np.float32) for a in (Wk, Wq, Wv, Wp))
    bk, bq, bv, bp = (np.asarray(a, dtype=np.float32) for a in (bk, bq, bv, bp))

    mask01 = np.where(np.tril(np.ones((128, 128), dtype=bool)).T, 1.0, 0.0).astype(NP_BF16)
    xT_b = [np.ascontiguousarray(x[b].T).astype(NP_BF16) for b in range(B)]
    in_maps = []
    for c in range(N_CORES):
        b, half = c // 2, c % 2
        hs = half * MPC
        in_maps.append({
            "xT": xT_b[b],
            "wqT": np.ascontiguousarray(Wq[hs:hs + MPC, :].T).astype(NP_BF16),
            "wkT": np.ascontiguousarray(Wk[hs:hs + MPC, :].T).astype(NP_BF16),
            "wvT": np.ascontiguousarray(Wv[hs:hs + MPC, :].T).astype(NP_BF16),
            "wpT": np.ascontiguousarray(Wp[:, hs:hs + MPC].T).astype(NP_BF16),
            # per-partition column layout for the DVE-side bias
            "bqc": np.ascontiguousarray(bq[hs:hs + MPC].reshape(MT, 128).T).astype(np.float32),
            "bkc": np.ascontiguousarray(bk[hs:hs + MPC].reshape(MT, 128).T).astype(np.float32),
            "bv": bv[hs:hs + MPC].reshape(1, MPC).astype(np.float32),
            "bp": (bp if half == 0 else np.zeros_like(bp)).reshape(1, C).astype(np.float32),
            "mask01": mask01,
        })
    return in_maps


def kernel(x, Wk, bk, Wq, bq, Wv, bv, Wp, bp, **run_kwargs):
    in_maps = make_in_maps(x, Wk, bk, Wq, bq, Wv, bv, Wp, bp)
    nc = _get_nc()
    res = run_bass_kernel_spmd(nc, in_maps, core_ids=list(range(N_CORES)), **run_kwargs)
    out = np.empty((B, T, C), dtype=np.float32)
    for b in range(B):
        out[b] = (res.results[2 * b]["out"].astype(np.float32)
                  + res.results[2 * b + 1]["out"].astype(np.float32))
    if run_kwargs:
        kernel.last_results = res
    return out


# revision 31
# speedup vs baseline: 1.0018x; 1.0018x over previous
"""Causal self-attention (B=4, T=2048, C=1024, H=16) on 8 NeuronCores.

Sharding: core c handles batch b = c//2 and head-half half = c%2 (8 heads,
512 channels). QKV projections are column-parallel, output projection is
row-parallel (Megatron); the two per-batch bf16 output partials are summed
on host in fp32.

All matmul operands are bf16 (fp32 PSUM accumulate): same 1 cycle/row PE
stream rate as fp32r but with fast-weight-load, 2-4x DVE element ops, and
half the DMA/SBUF footprint. Tolerance is 2e-2; bf16 end-to-end measures
~1e-3.

Per-core pipeline:
  phase 1: qT/kT = W @ xT (weight-stationary, [m, t] layout; bias applied
           free on the ACT psum-evacuation via a per-partition bias),
           v = xT.T @ WvT ([t, m] layout; bias folded into the DVE
           evacuation) with a constant ones-column per head (vAug) so the
           attention matmul also produces the softmax denominator.
  phase 2: processes HEAD PAIRS (2p, 2p+1) = partitions 0-63/64-127 of the
           kT/qT/yT tile p, in 512-query chunks. The hd=64 score matmuls
           run as two concurrent 64x128 PE row tiles (T0/T8), halving score
           time; the AV matmul splits its 128-key contraction into T0
           (keys 0-63 -> psyA) and T8 (keys 64-127 -> psyB) so the whole
           phase stays in 64-row tiling mode (no PE drains). P = exp(S^T)
           via one strided ACT instruction per key block covering both
           heads' column spans; 0/1 mask multiply on diagonal blocks.
           Softmax uses a fixed max of 0 (scores are ~N(0,1)); the
           denominator is row 64 of psy. psyA+psyB merge on the DVE
           evacuation; normalize via reciprocal + ones-broadcast +
           multiply, writing yT straight into SBUF.
  phase 3: out_partial = yT.T @ WpT (bias folded into the DVE evacuation;
           bp on half==0 cores only), bf16 output.

Benching: with bench_loops and all three phases selected, ONE hardware
For_i loop wraps the whole pipeline so consecutive iterations overlap
(phase 2's scalar-engine exp burst hides under phase 3 / next phase 1 PE
work). PSUM budget (8 banks): ps_s 2x[128,1024]=4 shared by all phases'
matmul groups + psyA/psyB [65,1024] = 2+2. The PE streams ~1 col/cycle at
an effective ~1.2 GHz in this environment (measured; the warm 2.4 GHz
state never engages), so total PE columns (~471k/core/iter) set the
floor.
"""

import sys
import types

import numpy as np
from contextlib import ExitStack

import ml_dtypes

import concourse.bass as bass
import concourse.mybir as mybir
import concourse.tile as tile
from concourse import bacc
from concourse.bass_utils import run_bass_kernel_spmd

# If the environment sets BASS_TRACE but ships only the antenv stub (no
# axon_hooks), run_bass_kernel_spmd would crash on import. Provide the
# graceful "no hook registered" fallback only when the real module is absent.
try:  # pragma: no cover
    import antenv.axon_hooks  # noqa: F401
except ImportError:  # pragma: no cover
    import antenv

    _stub = types.ModuleType("antenv.axon_hooks")
    _stub.get_axon_ntff_profile_hook = lambda: None
    sys.modules["antenv.axon_hooks"] = _stub
    antenv.axon_hooks = _stub

F32 = mybir.dt.float32
BF16 = mybir.dt.bfloat16
EXP = mybir.ActivationFunctionType.Exp
IDENT = mybir.ActivationFunctionType.Identity
NP_BF16 = ml_dtypes.bfloat16

B, T, C, H = 4, 2048, 1024, 16
HD = C // H              # 64 head dim
N_CORES = 8
HPC = H // 2             # 8 heads per core
MPC = C // 2             # 512 channels per core
MT = MPC // 128          # 4 m-tiles per core
CT = C // 128            # 8 contraction tiles
TC = T // 512            # 4 t-chunks
TT = T // 128            # 16 t-tiles
SCALE = float(1.0 / np.sqrt(HD))

_CACHE = {}


def _build(bench_loops=None, phases=(1, 2, 3)):
    import contextlib

    def _rep(cond=True):
        return tc.For_i(0, bench_loops, 1) if (bench_loops and cond) else contextlib.nullcontext()

    fused = bool(bench_loops) and set(phases) == {1, 2, 3}

    nc = bacc.Bacc()
    xT = nc.declare_dram_parameter("xT", [C, T], BF16, isOutput=False)
    wqT = nc.declare_dram_parameter("wqT", [C, MPC], BF16, isOutput=False)
    wkT = nc.declare_dram_parameter("wkT", [C, MPC], BF16, isOutput=False)
    wvT = nc.declare_dram_parameter("wvT", [C, MPC], BF16, isOutput=False)
    wpT = nc.declare_dram_parameter("wpT", [MPC, C], BF16, isOutput=False)
    bqc = nc.declare_dram_parameter("bqc", [128, MT], F32, isOutput=False)
    bkc = nc.declare_dram_parameter("bkc", [128, MT], F32, isOutput=False)
    bv = nc.declare_dram_parameter("bv", [1, MPC], F32, isOutput=False)
    bp = nc.declare_dram_parameter("bp", [1, C], F32, isOutput=False)
    mask01d = nc.declare_dram_parameter("mask01", [128, 128], BF16, isOutput=False)
    outp = nc.declare_dram_parameter("out", [T, C], BF16, isOutput=True)

    with tile.TileContext(nc) as tc:
        with ExitStack() as ctx:
            persist = ctx.enter_context(tc.tile_pool(name="persist", bufs=1))
            pool_p = ctx.enter_context(tc.tile_pool(name="pool_p", bufs=2))
            # PSUM budget (8 banks): ps_s 2x[128,1024]=4 shared by all phases'
            # matmul groups; psyA+psyB [65,1024] = 2+2.
            ps_s = ctx.enter_context(tc.tile_pool(name="ps_s", bufs=2, space="PSUM"))
            ps_y = ctx.enter_context(tc.tile_pool(name="ps_y", bufs=1, space="PSUM"))

            # ---- constants / small tensors ----
            mask01_sb = persist.tile([128, 128], BF16, name="mask01_sb")
            nc.sync.dma_start(out=mask01_sb, in_=mask01d[:, :])
            bqc_sb = persist.tile([128, MT], F32, name="bqc_sb")
            nc.sync.dma_start(out=bqc_sb, in_=bqc[:, :])
            bkc_sb = persist.tile([128, MT], F32, name="bkc_sb")
            nc.sync.dma_start(out=bkc_sb, in_=bkc[:, :])
            bv_sb = persist.tile([1, MPC], F32, name="bv_sb")
            nc.sync.dma_start(out=bv_sb, in_=bv[:, :])
            bp_sb = persist.tile([1, C], F32, name="bp_sb")
            nc.sync.dma_start(out=bp_sb, in_=bp[:, :])
            ones8 = persist.tile([128, 8], BF16, name="ones8")
            nc.vector.memset(ones8, 1.0)
            # biases broadcast across partitions once (f32, matching the psum
            # operand dtype); folded into the DVE evacuations instead of
            # burning PE columns on rank-1 matmuls
            bv_bc = persist.tile([128, MPC], F32, name="bv_bc")
            nc.gpsimd.partition_broadcast(bv_bc, bv_sb)
            bp_bc = persist.tile([128, C], F32, name="bp_bc")
            nc.gpsimd.partition_broadcast(bp_bc, bp_sb)

            # ---- persistent activations ----
            qT_sb = [persist.tile([128, T], BF16, name=f"qT{m}") for m in range(MT)]
            kT_sb = [persist.tile([128, T], BF16, name=f"kT{m}") for m in range(MT)]
            vAug = [persist.tile([128, HPC * (HD + 1)], BF16, name=f"vAug{t}") for t in range(TT)]
            yT_sb = [persist.tile([128, T], BF16, name=f"yT{m}") for m in range(MT)]

            # constant ones-column of vAug, written once
            for t_ in range(TT):
                va = vAug[t_].rearrange("p (h w) -> p h w", w=HD + 1)
                nc.vector.tensor_copy(va[:, :, HD], ones8)

            if 1 not in phases:
                # bench-only: initialize phase-1 products with arbitrary finite data
                for m in range(MT):
                    nc.sync.dma_start(out=qT_sb[m], in_=xT[0:128, :])
                    nc.sync.dma_start(out=kT_sb[m], in_=xT[128:256, :])
                for t_ in range(TT):
                    va = vAug[t_].rearrange("p (h w) -> p h w", w=HD + 1)
                    nc.sync.dma_start(out=va[:, :, 0:HD],
                                      in_=xT[0:128, 0:HPC * HD].rearrange("p (h w) -> p h w", w=HD))
            if 3 in phases and 2 not in phases:
                for m in range(MT):
                    nc.sync.dma_start(out=yT_sb[m], in_=xT[0:128, :])

            # ---- weights in SBUF (loaded once, outside any bench loop) ----
            pool_w = ctx.enter_context(tc.tile_pool(name="pool_w", bufs=1))
            if 1 in phases:
                wq_t = [pool_w.tile([128, MPC], BF16, name=f"wq{c}") for c in range(CT)]
                wk_t = [pool_w.tile([128, MPC], BF16, name=f"wk{c}") for c in range(CT)]
                wv_t = [pool_w.tile([128, MPC], BF16, name=f"wv{c}") for c in range(CT)]
                for c in range(CT):
                    nc.sync.dma_start(out=wq_t[c], in_=wqT[c * 128:(c + 1) * 128, :])
                    nc.sync.dma_start(out=wk_t[c], in_=wkT[c * 128:(c + 1) * 128, :])
                    nc.sync.dma_start(out=wv_t[c], in_=wvT[c * 128:(c + 1) * 128, :])
            if 3 in phases:
                wp_t = [pool_w.tile([128, C], BF16, name=f"wp{m}") for m in range(MT)]
                for m in range(MT):
                    nc.sync.dma_start(out=wp_t[m], in_=wpT[m * 128:(m + 1) * 128, :])

            pool_xs = ctx.enter_context(tc.tile_pool(name="pool_xs", bufs=1))
            pool_p2 = ctx.enter_context(tc.tile_pool(name="pool_p2", bufs=3))
            pool_o = ctx.enter_context(tc.tile_pool(name="pool_o", bufs=1))

            # ================= phase bodies =================
            def body1():
                for tch in range(TC):
                    t0 = tch * 512
                    xs = []
                    for c in range(CT):
                        x_ = pool_xs.tile([128, 512], BF16, name=f"xs_{tch}_{c}", tag="xs", bufs=12)
                        nc.sync.dma_start(out=x_, in_=xT[c * 128:(c + 1) * 128, t0:t0 + 512])
                        xs.append(x_)
                    # qT / kT (weight-stationary): psum[m 128, t 512]; bias is
                    # per-partition so the DVE evacuation applies it for free.
                    for wt, bias_col, dst in ((wq_t, bqc_sb, qT_sb), (wk_t, bkc_sb, kT_sb)):
                        for m in range(MT):
                            ps = ps_s.tile([128, 512], F32, name=f"ps_{tch}_{id(wt)}_{m}",
                                           tag="pss", padded_shape=[128, 1024])
                            for c in range(CT):
                                nc.tensor.matmul(ps, wt[c][:, m * 128:(m + 1) * 128], xs[c],
                                                 start=(c == 0), stop=(c == CT - 1))
                            nc.scalar.activation(out=dst[m][:, t0:t0 + 512], in_=ps,
                                                 func=IDENT, bias=bias_col[:, m:m + 1])
                    # v (x-stationary): psum[t 128, m 512] -> vAug; bias folded
                    # into the DVE evacuation via the broadcast bias tile.
                    for tt in range(4):
                        tg = tch * 4 + tt
                        ps = ps_s.tile([128, MPC], F32, name=f"psv_{tg}",
                                       tag="pss", padded_shape=[128, 1024])
                        for c in range(CT):
                            nc.tensor.matmul(ps, xs[c][:, tt * 128:(tt + 1) * 128], wv_t[c],
                                             start=(c == 0), stop=(c == CT - 1))
                        va = vAug[tg].rearrange("p (h w) -> p h w", w=HD + 1)
                        nc.vector.tensor_add(va[:, :, 0:HD],
                                             ps.rearrange("p (h w) -> p h w", w=HD),
                                             bv_bc.rearrange("p (h w) -> p h w", w=HD))

            def body2():
                # Head PAIRS (2p, 2p+1) = partitions 0-63 / 64-127 of kT/qT tile
                # p. Scores run as two concurrent 64x128 row tiles (T0/T8); AV
                # splits the 128-key contraction into T0 (keys 0-63 -> psyA) and
                # T8 (keys 64-127 -> psyB), also concurrent. All phase-2 matmuls
                # stay in 64-row tiling mode (no PE drains inside the loop).
                # Query chunks of 512: psum layout per chunk is
                #   ps_pair[128, 1024]: cols 0:512 even head (bank 0),
                #                       cols 512:1024 odd head (bank 1)
                #   psyA/psyB[65, 1024]: cols 0:512 even (bank), 512:1024 odd
                #   (bank) — one start/stop per bank per accumulation round.
                W = 512
                for p in range(MT):
                    hE, hO = 2 * p, 2 * p + 1
                    kT_h = kT_sb[p]
                    qT_h = qT_sb[p]
                    for qc in range(T // W):
                        q0 = qc * W
                        jmax = 4 * qc + 3
                        psyA = ps_y.tile([HD + 1, 1024], F32, name=f"psyA_{p}_{qc}", tag="psyA")
                        psyB = ps_y.tile([HD + 1, 1024], F32, name=f"psyB_{p}_{qc}", tag="psyB")
                        pend = None

                        def _emit_y(j, P, lo):
                            # row groups alternate so each LDWEIGHTS hides under
                            # the other tile's in-flight matmul
                            for hh, co in ((hE, 0), (hO, W)):
                                for half, psy in ((0, psyA), (64, psyB)):
                                    nc.tensor.matmul(
                                        psy[:, co + lo:co + W],
                                        vAug[j][half:half + 64, hh * (HD + 1):(hh + 1) * (HD + 1)],
                                        P[half:half + 64, co + lo:co + W],
                                        start=(j == 0), stop=(j == jmax),
                                        tile_position=(half, 0))

                        for j in range(jmax + 1):
                            lo = max(0, 128 * j - q0)
                            pss = ps_s.tile([128, 1024], F32, name=f"pss_{p}_{qc}_{j}", tag="pss")
                            for half, co in ((0, 0), (64, W)):
                                nc.tensor.matmul(
                                    pss[:, co + lo:co + W],
                                    kT_h[half:half + HD, j * 128:(j + 1) * 128],
                                    qT_h[half:half + HD, q0 + lo:q0 + W],
                                    start=True, stop=True, tile_position=(half, 0))
                            P = pool_p.tile([128, 1024], BF16, name=f"P_{p}_{qc}_{j}", tag="P", bufs=3)
                            # one strided activation covers both heads' spans
                            pv = pss.rearrange("p (g w) -> p g w", w=W)
                            Pv = P.rearrange("p (g w) -> p g w", w=W)
                            nc.scalar.activation(out=Pv[:, :, lo:W], in_=pv[:, :, lo:W],
                                                 func=EXP, scale=SCALE)
                            if 128 * j >= q0:  # diagonal block: in-tile causal mask
                                for co in (0, W):
                                    nc.vector.tensor_mul(P[:, co + lo:co + lo + 128],
                                                         P[:, co + lo:co + lo + 128], mask01_sb)
                            if pend is not None:
                                _emit_y(*pend)
                            pend = (j, P, lo)
                        _emit_y(*pend)
                        # merge the two key-half accumulators: copy psyA out,
                        # then accumulate psyB in place (one PSUM operand per
                        # tensor_tensor — the ISA forbids two), normalize, and
                        # write yT straight into SBUF.
                        yu = pool_p2.tile([HD + 1, 1024], BF16, name=f"yu_{p}_{qc}", tag="yu")
                        nc.vector.tensor_copy(yu, psyA)
                        nc.vector.tensor_add(yu, yu, psyB)
                        r16 = pool_p2.tile([1, 1024], BF16, name=f"r16_{p}_{qc}", tag="r16")
                        with nc.allow_low_precision(reason="denominator recip; 2e-2 tol"):
                            nc.vector.reciprocal(r16, yu[HD:HD + 1, :])
                        rb = pool_p2.tile([HD, 1024], BF16, name=f"rb_{p}_{qc}", tag="rb")
                        nc.gpsimd.partition_broadcast(rb, r16)
                        for so, co in ((0, 0), (64, W)):
                            nc.vector.tensor_mul(yT_sb[p][so:so + HD, q0:q0 + W],
                                                 yu[0:HD, co:co + W], rb[:, co:co + W])

            def body3():
                for tt in range(TT):
                    for nch in range(2):
                        n0 = nch * 512
                        ps = ps_s.tile([128, 512], F32, name=f"pso_{tt}_{nch}",
                                       tag="pss", padded_shape=[128, 1024])
                        for m in range(MT):
                            nc.tensor.matmul(ps, yT_sb[m][:, tt * 128:(tt + 1) * 128],
                                             wp_t[m][:, n0:n0 + 512],
                                             start=(m == 0), stop=(m == MT - 1))
                        o_sb = pool_o.tile([128, 512], BF16, name=f"o_{tt}_{nch}", tag="o", bufs=3)
                        nc.vector.tensor_add(o_sb, ps, bp_bc[:, n0:n0 + 512])
                        nc.sync.dma_start(out=outp[tt * 128:(tt + 1) * 128, n0:n0 + 512], in_=o_sb)

            # ================= emission =================
            if fused:
                with _rep():
                    body1()
                    body2()
                    body3()
            else:
                if 1 in phases:
                    with _rep():
                        body1()
                if 2 in phases:
                    with _rep():
                        body2()
                if 3 in phases:
                    with _rep():
                        body3()
    nc.finalize()
    return nc


def _get_nc(bench_loops=None, phases=(1, 2, 3)):
    key = ("nc", bench_loops, tuple(phases))
    if key not in _CACHE:
        _CACHE[key] = _build(bench_loops, phases)
    return _CACHE[key]


def make_in_maps(x, Wk, bk, Wq, bq, Wv, bv, Wp, bp):
    x = np.asarray(x, dtype=np.float32)
    Wk, Wq, Wv, Wp = (np.asarray(a, dtype=np.float32) for a in (Wk, Wq, Wv, Wp))
    bk, bq, bv, bp = (np.asarray(a, dtype=np.float32) for a in (bk, bq, bv, bp))

    mask01 = np.where(np.tril(np.ones((128, 128), dtype=bool)).T, 1.0, 0.0).astype(NP_BF16)
    xT_b = [np.ascontiguousarray(x[b].T).astype(NP_BF16) for b in range(B)]
    in_maps = []
    for c in range(N_CORES):
        b, half = c // 2, c % 2
        hs = half * MPC
        in_maps.append({
            "xT": xT_b[b],
            "wqT": np.ascontiguousarray(Wq[hs:hs + MPC, :].T).astype(NP_BF16),
            "wkT": np.ascontiguousarray(Wk[hs:hs + MPC, :].T).astype(NP_BF16),
            "wvT": np.ascontiguousarray(Wv[hs:hs + MPC, :].T).astype(NP_BF16),
            "wpT": np.ascontiguousarray(Wp[:, hs:hs + MPC].T).astype(NP_BF16),
            # per-partition column layout for the DVE-side bias
            "bqc": np.ascontiguousarray(bq[hs:hs + MPC].reshape(MT, 128).T).astype(np.float32),
            "bkc": np.ascontiguousarray(bk[hs:hs + MPC].reshape(MT, 128).T).astype(np.float32),
            "bv": bv[hs:hs + MPC].reshape(1, MPC).astype(np.float32),
            "bp": (bp if half == 0 else np.zeros_like(bp)).reshape(1, C).astype(np.float32),
            "mask01": mask01,
        })
    return in_maps


def kernel(x, Wk, bk, Wq, bq, Wv, bv, Wp, bp, **run_kwargs):
    in_maps = make_in_maps(x, Wk, bk, Wq, bq, Wv, bv, Wp, bp)
    nc = _get_nc()
    res = run_bass_kernel_spmd(nc, in_maps, core_ids=list(range(N_CORES)), **run_kwargs)
    out = np.empty((B, T, C), dtype=np.float32)
    for b in range(B):
        out[b] = (res.results[2 * b]["out"].astype(np.float32)
                  + res.results[2 * b + 1]["out"].astype(np.float32))
    if run_kwargs:
        kernel.last_results = res
    return out
